# revision 15
# baseline (speedup 1.0000x reference)
"""GIN encoder (5-layer GNN + BN + global pooling) on 8 TRN2 NeuronCores.

kernel(**inputs) takes FULL inputs, returns FULL [8192, 128] output.

v2 design (SPMD, one bass program, per-core node shards of 25000):
- Aggregation: per-64-dst-slot tile, edges (incl. self-loops) sorted by dst,
  chunked into <=128-row groups; gathered from a bf16 node table via BATCHED
  indirect DMA (one DMA per 32-tile block, ~0.34ns/row on GpSimd), selection
  matrices generated ON-CHIP (iota + is_equal against dst-slot values, 8
  chunks per vector op) instead of streamed from HBM.
- Table double-buffered (A/B per layer) so the AllGather of layer L's output
  (4 chunks, fired as 512-node blocks complete) overlaps layer L's tail.
- MLP1 in float32r (fp32 precision at bf16 speed, N=512), BN-shift folded as
  rank-1 v*deg' into the MLP1 PSUM group (v = W1^T t); MLP2 node-major bf16
  with ones x b2 rank-1; pooling from an f32 copy of z; BN stats via Gram
  diagonal + pooled ones column, AllReduced; per-layer pool results scaled by
  s_L*lcw_L and accumulated on-chip into pacc; host only overlap-adds windows
  and adds the cnt*(sum lcw_L t_L + lc_b) term.
"""
import numpy as np
import ml_dtypes

from concourse import bass, bacc, tile, mybir
from concourse import bass_utils

N_NODES = 200000
N_EDGES = 400000
N_FEAT = 78
DIM = 128
N_LAYERS = 5
N_GRAPHS = 8192
BN_EPS = 1e-5
NC = 8
NLOC = N_NODES // NC           # 25000
W64 = 128                          # dst-tile width (historical name)
NT64 = (NLOC + W64 - 1) // W64     # 196 (last tile 40)
NT128 = (NLOC + 127) // 128        # 196 (last tile 40)
NBLK = (NLOC + 511) // 512         # 49 MLP blocks (last 424)
GB_MB = 4                          # MLP blocks per gather block
NGB = (NBLK + GB_MB - 1) // GB_MB  # 13 gather blocks
# AllGather chunks (rows per core)
AGR = [6144, 6144, 6144, NLOC - 3 * 6144]           # [6144,6144,6144,6568]
AGS = [0, 6144, 12288, 18432]                       # local row starts
AGB = [0, 8 * 6144, 16 * 6144, 24 * 6144]           # table base offsets
AG_FIRE = [11, 23, 35, NBLK - 1]                    # after these MLP blocks
OOB = 1 << 30
SGRP = 8                            # chunks per S-generation vector op

f32 = mybir.dt.float32
f32r = mybir.dt.float32r
bf16 = mybir.dt.bfloat16
i32 = mybir.dt.int32
Relu = mybir.ActivationFunctionType.Relu
Copy = mybir.ActivationFunctionType.Copy
Sqrt = mybir.ActivationFunctionType.Sqrt
ADD = mybir.AluOpType.add
MULT = mybir.AluOpType.mult
SUB = mybir.AluOpType.subtract
ISEQ = mybir.AluOpType.is_equal

_CACHE = {}
_LAST_RES = None


def _table_pos(n):
    """Map global node id -> row in the AG-chunked table layout."""
    r = n // NLOC
    j = n % NLOC
    cc = np.minimum(j // 6144, 3)
    base = np.asarray(AGB, np.int64)[cc]
    rows = np.asarray(AGR, np.int64)[cc]
    start = np.asarray(AGS, np.int64)[cc]
    return base + r * rows + (j - start)


def _prep(src, dst, batch):
    order = np.argsort(dst, kind="stable")
    src_s = src[order]
    dst_s = dst[order]

    cores = []
    cnt64 = np.zeros((NC, NT64), np.int64)
    for c in range(NC):
        lo = c * NLOC
        a = np.searchsorted(dst_s, lo)
        b = np.searchsorted(dst_s, lo + NLOC)
        es = src_s[a:b]
        ed = dst_s[a:b] - lo
        cnt64[c] = np.bincount(ed // W64, minlength=NT64)
        cores.append((es, ed))

    T64 = tuple(int(v) for v in
                np.ceil(cnt64.max(axis=0) / 128).astype(np.int64))
    cstart = np.concatenate([[0], np.cumsum(T64)]).astype(np.int64)
    NCH = int(cstart[-1])

    PW = 0
    glo_all = []
    for c in range(NC):
        bseg = batch[c * NLOC:(c + 1) * NLOC]
        glo = np.zeros(NT128, np.int64)
        for t in range(NT128):
            seg = bseg[t * 128: min((t + 1) * 128, NLOC)]
            glo[t] = seg[0]
            PW = max(PW, int(seg[-1] - seg[0] + 1))
        glo_all.append(glo)
    PW += 1  # ones column

    percore = []
    for c in range(NC):
        es, ed = cores[c]
        tpe = ed // W64                           # tile per edge
        tile_first = np.searchsorted(ed, np.arange(NT64) * W64)
        rank = np.arange(len(ed)) - tile_first[tpe]
        col = cstart[tpe] + rank // 128
        row = rank % 128
        idx = np.full((128, NCH), OOB, np.int32)
        dsl = np.full((128, NCH), -1.0, np.float32)
        idx[row, col] = _table_pos(es).astype(np.int32)
        dsl[row, col] = (ed - tpe * W64).astype(np.float32)

        bseg = batch[c * NLOC:(c + 1) * NLOC]
        glo = glo_all[c]
        Sp = np.zeros((128, NT128 * PW), np.float32)
        for t in range(NT128):
            n0, n1 = t * 128, min(t * 128 + 128, NLOC)
            p = np.arange(n1 - n0)
            Sp[p, t * PW + (bseg[n0:n1] - glo[t])] = 1.0
            Sp[p, t * PW + PW - 1] = 1.0

        degp = np.zeros((1, NBLK * 512), np.float32)
        degp[0, :NLOC] = np.bincount(
            ed, minlength=NLOC).astype(np.float32) + 1.0  # +1 for self

        percore.append(dict(
            idx=idx, dsl=dsl.astype(ml_dtypes.bfloat16),
            Sp=Sp, degp=degp, glo=glo))

    return percore, T64, PW


def _build(T64, PW):
    nc = bacc.Bacc("TRN2", target_bir_lowering=False, debug=False,
                   num_devices=NC)
    cstart = np.concatenate([[0], np.cumsum(T64)]).astype(np.int64)
    NCH = int(cstart[-1])
    L5 = N_LAYERS
    RG = [list(range(NC))]

    # gather-block chunk ranges
    gb_rng = []
    for gb in range(NGB):
        mb0 = gb * GB_MB
        mb1 = min(mb0 + GB_MB, NBLK)
        t0 = mb0 * 4
        t1 = min(mb1 * 4, NT64)
        gb_rng.append((int(cstart[t0]), int(cstart[t1])))
    GMAX = max(c1 - c0 for c0, c1 in gb_rng)

    xT_in = nc.dram_tensor("xT", [N_FEAT, NLOC], f32, kind="ExternalInput")
    idx_in = nc.dram_tensor("idx", [128, NCH], i32, kind="ExternalInput")
    dsl_in = nc.dram_tensor("dsl", [128, NCH], bf16, kind="ExternalInput")
    Sp_in = nc.dram_tensor("Sp", [128, NT128 * PW], f32, kind="ExternalInput")
    degp_in = nc.dram_tensor("degp", [1, NBLK * 512], f32r,
                             kind="ExternalInput")
    If_in = nc.dram_tensor("If", [128, 128], f32, kind="ExternalInput")
    iota_in = nc.dram_tensor("iota", [128, SGRP * W64], bf16,
                             kind="ExternalInput")
    ib2bc_in = nc.dram_tensor("ib2bc", [128, 128], f32, kind="ExternalInput")
    lcw_in = nc.dram_tensor("lcwbc", [128, 8], f32, kind="ExternalInput")
    iw1_in = nc.dram_tensor("iw1", [N_FEAT, DIM], f32, kind="ExternalInput")
    ib1_in = nc.dram_tensor("ib1", [DIM, 1], f32, kind="ExternalInput")
    iw2_in = nc.dram_tensor("iw2", [DIM, DIM], f32, kind="ExternalInput")
    w1_in = nc.dram_tensor("w1", [DIM, L5 * DIM], f32, kind="ExternalInput")
    b1T_in = nc.dram_tensor("b1T", [DIM, L5], f32, kind="ExternalInput")
    w2_in = nc.dram_tensor("w2", [DIM, L5 * DIM], f32, kind="ExternalInput")
    b2r_in = nc.dram_tensor("b2r", [1, L5 * DIM], f32, kind="ExternalInput")
    gamT_in = nc.dram_tensor("gamT", [DIM, L5], f32, kind="ExternalInput")
    betT_in = nc.dram_tensor("betT", [DIM, L5], f32, kind="ExternalInput")

    pacc_out = nc.dram_tensor("pacc", [128, NT128 * PW], f32,
                              kind="ExternalOutput")
    st_out = nc.dram_tensor("st", [L5 * 2, DIM], f32, kind="ExternalOutput")

    tableA = nc.dram_tensor("tableA", [N_NODES, DIM], bf16, kind="Internal",
                            addr_space="Shared")
    tableB = nc.dram_tensor("tableB", [N_NODES, DIM], bf16, kind="Internal",
                            addr_space="Shared")
    zbufc = [nc.dram_tensor(f"zbufc{i}", [AGR[i], DIM], bf16, kind="Internal")
             for i in range(4)]
    ar_in = nc.dram_tensor("ar_in", [DIM, 2], f32, kind="Internal")
    ar_out = nc.dram_tensor("ar_out", [DIM, 2], f32, kind="Internal",
                            addr_space="Shared")
    v_dram = nc.dram_tensor("v_dram", [1, DIM], f32r, kind="Internal")

    with tile.TileContext(nc) as tc:
        with tc.tile_pool(name="const", bufs=1) as cp, \
             tc.tile_pool(name="gp", bufs=2) as gp, \
             tc.tile_pool(name="slp", bufs=2) as slp, \
             tc.tile_pool(name="sgp", bufs=3) as sgp, \
             tc.tile_pool(name="z1p", bufs=2) as z1p, \
             tc.tile_pool(name="y1p", bufs=2) as y1p, \
             tc.tile_pool(name="y1ip", bufs=2) as y1ip, \
             tc.tile_pool(name="zsp", bufs=3) as zsp, \
             tc.tile_pool(name="z32p", bufs=3) as z32p, \
             tc.tile_pool(name="xbp", bufs=2) as xbp, \
             tc.tile_pool(name="psU", bufs=2, space="PSUM") as psU, \
             tc.tile_pool(name="psY", bufs=2, space="PSUM") as psY, \
             tc.tile_pool(name="psZ", bufs=2, space="PSUM") as psZ, \
             tc.tile_pool(name="psP", bufs=1, space="PSUM") as psP, \
             tc.tile_pool(name="psG", bufs=1, space="PSUM") as psG:

            def ld(shape, dt_, src_ap, name):
                t_ = cp.tile(shape, dt_, name=name)
                nc.sync.dma_start(t_[:], src_ap)
                return t_

            idx_t = ld([128, NCH], i32, idx_in[:], "idx_t")
            dsl_t = ld([128, NCH], bf16, dsl_in[:], "dsl_t")
            Sp_t = ld([128, NT128 * PW], f32, Sp_in[:], "Sp_t")
            degp_t = ld([1, NBLK * 512], f32r, degp_in[:], "degp_t")
            If_t = ld([128, 128], f32, If_in[:], "If_t")
            Ib_t = cp.tile([128, 128], bf16, name="Ib_t")
            nc.vector.tensor_copy(Ib_t[:], If_t[:])
            iota_t = ld([128, SGRP * W64], bf16, iota_in[:], "iota_t")
            ib2bc_t = ld([128, 128], f32, ib2bc_in[:], "ib2bc_t")
            lcw_t = ld([128, 8], f32, lcw_in[:], "lcw_t")
            iw1_t = ld([N_FEAT, DIM], f32, iw1_in[:], "iw1_t")
            iw1r = cp.tile([N_FEAT, DIM], f32r, name="iw1r")
            nc.vector.tensor_copy(iw1r[:], iw1_t[:])
            ib1_t = ld([DIM, 1], f32, ib1_in[:], "ib1_t")
            iw2_t = ld([DIM, DIM], f32, iw2_in[:], "iw2_t")
            w1_t = ld([DIM, L5 * DIM], f32, w1_in[:], "w1_t")
            b1T_t = ld([DIM, L5], f32, b1T_in[:], "b1T_t")
            w2_t = ld([DIM, L5 * DIM], f32, w2_in[:], "w2_t")
            b2r_t = ld([1, L5 * DIM], f32, b2r_in[:], "b2r_t")
            gamT_t = ld([DIM, L5], f32, gamT_in[:], "gamT_t")
            betT_t = ld([DIM, L5], f32, betT_in[:], "betT_t")

            w2b = cp.tile([DIM, L5 * DIM], bf16, name="w2b")
            nc.vector.tensor_copy(w2b[:], w2_t[:])
            b2rb = cp.tile([1, L5 * DIM], bf16, name="b2rb")
            nc.vector.tensor_copy(b2rb[:], b2r_t[:])
            ones_r = cp.tile([1, 128], bf16, name="ones_r")
            nc.vector.memset(ones_r[:], 1.0)

            w1f = cp.tile([DIM, DIM], f32r, name="w1f")
            vrow = cp.tile([1, DIM], f32r, name="vrow")
            pall = cp.tile([128, NT128 * PW], f32, name="pall")
            pacc = cp.tile([128, NT128 * PW], f32, name="pacc")
            nc.vector.memset(pacc[:], 0.0)
            zsum_t = cp.tile([DIM, 1], f32, name="zsum_t")
            stats_sb = cp.tile([DIM, 2], f32, name="stats_sb")

            for _ in range(2):
                g0 = gp.tile([128, GMAX * 128], bf16, name="g", tag="g")
                nc.vector.memset(g0[:], 0.0)

            def write_zbuf(mb, zstage):
                """Write 512-node block mb from zstage [128, 512] bf16."""
                n0 = mb * 512
                bw = min(512, NLOC - n0)
                cc = min(n0 // 6144, 3)
                r0 = n0 - AGS[cc]
                nfull = bw // 128
                if nfull:
                    hbm = zbufc[cc].ap()[r0:r0 + nfull * 128, :].rearrange(
                        "(q p) f -> p q f", p=128)
                    sb = zstage[:, 0:nfull * 128].rearrange(
                        "p (q f) -> p q f", f=128)
                    nc.sync.dma_start(hbm, sb)
                rem = bw - nfull * 128
                if rem:
                    nc.sync.dma_start(
                        zbufc[cc].ap()[r0 + nfull * 128:r0 + bw, :],
                        zstage[:rem, nfull * 128:(nfull + 1) * 128])

            def fire_ag(mb, tout):
                if mb in AG_FIRE:
                    cc = AG_FIRE.index(mb)
                    nc.gpsimd.collective_compute(
                        "AllGather", mybir.AluOpType.bypass,
                        replica_groups=RG,
                        ins=[zbufc[cc].ap()],
                        outs=[tout.ap()[AGB[cc]:AGB[cc] + NC * AGR[cc], :]])

            # ---------------- ini embed -> zbufc/tableA ----------------
            for mb in range(NBLK):
                n0 = mb * 512
                bw = min(512, NLOC - n0)
                xb = xbp.tile([N_FEAT, 512], f32, name="xb", tag="xb")
                nc.sync.dma_start(xb[:, :bw], xT_in[:, n0:n0 + bw])
                xbr = xbp.tile([N_FEAT, 512], f32r, name="xbr", tag="xbr")
                nc.vector.tensor_copy(xbr[:, :bw], xb[:, :bw])
                yp = psY.tile([DIM, 512], f32, name="yp", tag="yp")
                nc.tensor.matmul(yp[:, :bw], lhsT=iw1r[:],
                                 rhs=xbr[:, :bw],
                                 start=True, stop=True)
                y1i = y1ip.tile([DIM, 512], f32, name="y1i", tag="y1i")
                nc.scalar.activation(y1i[:, :bw], yp[:, :bw], Relu,
                                     bias=ib1_t[:], scale=1.0)
                zstage = zsp.tile([128, 512], bf16, name="zs", tag="zs")
                for q in range((bw + 127) // 128):
                    cw = min(128, bw - q * 128)
                    zp = psZ.tile([128, DIM], f32, name="zp", tag="zp")
                    nc.tensor.matmul(zp[:cw, :],
                                     lhsT=y1i[:, q * 128:q * 128 + cw],
                                     rhs=iw2_t[:], start=True, stop=True)
                    nc.vector.tensor_tensor(
                        out=zstage[:cw, q * 128:(q + 1) * 128],
                        in0=zp[:cw, :], in1=ib2bc_t[:cw, :], op=ADD)
                write_zbuf(mb, zstage)
                fire_ag(mb, tableA)

            # ---------------- layers ----------------
            for L in range(L5):
                tin = tableA if L % 2 == 0 else tableB
                tout = tableB if L % 2 == 0 else tableA
                if L == 0:
                    nc.vector.tensor_copy(w1f[:], w1_t[:, 0:DIM])
                nc.vector.memset(zsum_t[:], 0.0)
                gram = psG.tile([128, 128], f32, name="gram", tag="gram")

                def issue_gather(gb):
                    c0, c1 = gb_rng[gb]
                    g = gp.tile([128, GMAX * 128], bf16, name="g", tag="g")
                    for ch in range(c0, c1):
                        nc.gpsimd.indirect_dma_start(
                            out=g[:, (ch - c0) * 128:(ch - c0 + 1) * 128],
                            out_offset=None, in_=tin[:],
                            in_offset=bass.IndirectOffsetOnAxis(
                                ap=idx_t[:, ch:ch + 1], axis=0),
                            bounds_check=N_NODES - 1, oob_is_err=False)
                    return g

                def read_self(gb):
                    n0 = gb * GB_MB * 512
                    bw = min(GB_MB * 512, NLOC - n0)
                    cc = min(n0 // 6144, 3)
                    r0 = n0 - AGS[cc]
                    sl = slp.tile([128, GB_MB * 512], bf16, name="sl",
                                  tag="sl")
                    nfull = bw // 128
                    if nfull:
                        nc.sync.dma_start(
                            sl[:, 0:nfull * 128].rearrange(
                                "p (q f) -> p q f", f=128),
                            zbufc[cc].ap()[r0:r0 + nfull * 128, :].rearrange(
                                "(q p) f -> p q f", p=128))
                    rem = bw - nfull * 128
                    if rem:
                        nc.sync.dma_start(
                            sl[:rem, nfull * 128:(nfull + 1) * 128],
                            zbufc[cc].ap()[r0 + nfull * 128:r0 + bw, :])
                    return sl

                g_cur = issue_gather(0)
                sl_cur = read_self(0)
                g_nxt = None
                sl_nxt = None
                for gb in range(NGB):
                    c0, c1 = gb_rng[gb]
                    if gb + 1 < NGB:
                        g_nxt = issue_gather(gb + 1)
                        sl_nxt = read_self(gb + 1)
                    sg_tiles = {}
                    for mb in range(gb * GB_MB, min((gb + 1) * GB_MB, NBLK)):
                        n0 = mb * 512
                        bw = min(512, NLOC - n0)
                        nsub = (bw + W64 - 1) // W64
                        z1b = z1p.tile([DIM, 512], f32r, name="z1b", tag="z1")
                        for s_ in range(nsub):
                            t64 = mb * 4 + s_
                            w64 = min(W64, NLOC - t64 * W64)
                            u = psU.tile([128, W64], f32, name="u", tag="u")
                            Tt = T64[t64]
                            for k in range(Tt):
                                ch = int(cstart[t64]) + k
                                grp = (ch - c0) // SGRP
                                if grp not in sg_tiles:
                                    gw = min(SGRP, (c1 - c0) - grp * SGRP)
                                    sg = sgp.tile([128, SGRP * W64], bf16,
                                                  name="sg", tag="sg")
                                    dslv = dsl_t[:, c0 + grp * SGRP:
                                                 c0 + grp * SGRP + gw]
                                    nc.vector.tensor_tensor(
                                        out=sg[:, :gw * W64].rearrange(
                                            "p (c j) -> p c j", j=W64),
                                        in0=iota_t[:, :gw * W64].rearrange(
                                            "p (c j) -> p c j", j=W64),
                                        in1=dslv.unsqueeze(2).broadcast_to(
                                            [128, gw, W64]),
                                        op=ISEQ)
                                    sg_tiles[grp] = sg
                                sg = sg_tiles[grp]
                                off = ((ch - c0) % SGRP) * W64
                                nc.tensor.matmul(
                                    u[:, :w64],
                                    lhsT=g_cur[:, (ch - c0) * 128:
                                               (ch - c0) * 128 + 128],
                                    rhs=sg[:, off:off + w64],
                                    start=(k == 0), stop=False)
                            toff = (t64 - gb * GB_MB * 4) * 128
                            nc.tensor.matmul(
                                u[:, :w64],
                                lhsT=sl_cur[:w64, toff:toff + 128],
                                rhs=Ib_t[:w64, :w64],
                                start=(Tt == 0), stop=True)
                            nc.vector.tensor_copy(
                                z1b[:, s_ * W64:s_ * W64 + w64], u[:, :w64])
                        # MLP1 (f32r) + BN-shift rank-1
                        yp = psY.tile([DIM, 512], f32, name="yp", tag="yp")
                        nc.tensor.matmul(yp[:, :bw],
                                         lhsT=w1f[:],
                                         rhs=z1b[:, :bw],
                                         start=True, stop=(L == 0))
                        if L > 0:
                            nc.tensor.matmul(
                                yp[:, :bw], lhsT=vrow[:],
                                rhs=degp_t[0:1, n0:n0 + bw],
                                start=False, stop=True)
                        y1b = y1p.tile([DIM, 512], bf16, name="y1b", tag="y1")
                        nc.scalar.activation(y1b[:, :bw], yp[:, :bw], Relu,
                                             bias=b1T_t[:, L:L + 1],
                                             scale=1.0)
                        zstage = zsp.tile([128, 512], bf16, name="zs",
                                          tag="zs")
                        for q in range((bw + 127) // 128):
                            ck = mb * 4 + q
                            cw = min(128, bw - q * 128)
                            zp = psZ.tile([128, DIM], f32, name="zp",
                                          tag="zp")
                            nc.tensor.matmul(
                                zp[:cw, :],
                                lhsT=y1b[:, q * 128:q * 128 + cw],
                                rhs=w2b[:, L * DIM:(L + 1) * DIM],
                                start=True, stop=False)
                            nc.tensor.matmul(
                                zp[:cw, :], lhsT=ones_r[:, :cw],
                                rhs=b2rb[:, L * DIM:(L + 1) * DIM],
                                start=False, stop=True)
                            zt32 = z32p.tile([128, DIM], f32, name="z32",
                                             tag="z32")
                            nc.vector.tensor_scalar(
                                out=zt32[:cw, :], in0=zp[:cw, :],
                                scalar1=0.0, scalar2=None,
                                op0=mybir.AluOpType.max)
                            nc.vector.tensor_copy(
                                zstage[:cw, q * 128:(q + 1) * 128],
                                zt32[:cw, :])
                            nc.tensor.matmul(
                                gram[:],
                                lhsT=zstage[:cw, q * 128:(q + 1) * 128],
                                rhs=zstage[:cw, q * 128:(q + 1) * 128],
                                start=(ck == 0), stop=(ck == NT128 - 1))
                            pout = psP.tile([128, PW], f32, name="pout",
                                            tag="pout")
                            nc.tensor.matmul(
                                pout[:], lhsT=zt32[:cw, :],
                                rhs=Sp_t[:cw, ck * PW:(ck + 1) * PW],
                                start=True, stop=True)
                            nc.scalar.activation(
                                pall[:, ck * PW:(ck + 1) * PW], pout[:],
                                Copy, scale=1.0)
                        write_zbuf(mb, zstage)
                        if L < L5 - 1:
                            fire_ag(mb, tout)
                    g_cur = g_nxt
                    sl_cur = sl_nxt

                # ---- stats + BN fold
                gmul = z32p.tile([128, 128], f32, name="gmul", tag="z32")
                nc.vector.tensor_tensor(out=gmul[:], in0=gram[:],
                                        in1=If_t[:], op=MULT)
                zsq = cp.tile([DIM, 1], f32, name=f"zsq{L}")
                nc.vector.tensor_reduce(out=zsq[:], in_=gmul[:], op=ADD,
                                        axis=mybir.AxisListType.X)
                # zsum: strided reduce over the ones columns of pall
                nc.vector.tensor_reduce(
                    out=zsum_t[:],
                    in_=pall[:].rearrange("p (t w) -> p w t", w=PW)
                    [:, PW - 1:PW, :].squeeze(1),
                    op=ADD, axis=mybir.AxisListType.X)
                nc.vector.tensor_copy(stats_sb[:, 0:1], zsum_t[:])
                nc.vector.tensor_copy(stats_sb[:, 1:2], zsq[:])
                nc.sync.dma_start(ar_in.ap()[:, :], stats_sb[:])
                nc.gpsimd.collective_compute(
                    "AllReduce", ADD, replica_groups=RG,
                    ins=[ar_in.ap()], outs=[ar_out.ap()])
                arst = cp.tile([DIM, 2], f32, name=f"arst{L}")
                nc.sync.dma_start(arst[:], ar_out.ap()[:, :])
                mean = cp.tile([DIM, 1], f32, name=f"mean{L}")
                nc.vector.tensor_scalar(out=mean[:], in0=arst[:, 0:1],
                                        scalar1=1.0 / N_NODES, scalar2=None,
                                        op0=MULT)
                ex2 = cp.tile([DIM, 1], f32, name=f"ex2{L}")
                nc.vector.tensor_scalar(out=ex2[:], in0=arst[:, 1:2],
                                        scalar1=1.0 / N_NODES, scalar2=None,
                                        op0=MULT)
                m2 = cp.tile([DIM, 1], f32, name=f"m2{L}")
                nc.vector.tensor_tensor(out=m2[:], in0=mean[:], in1=mean[:],
                                        op=MULT)
                var = cp.tile([DIM, 1], f32, name=f"var{L}")
                nc.vector.tensor_tensor(out=var[:], in0=ex2[:], in1=m2[:],
                                        op=SUB)
                vare = cp.tile([DIM, 1], f32, name=f"vare{L}")
                nc.vector.tensor_scalar(out=vare[:], in0=var[:],
                                        scalar1=BN_EPS, scalar2=None,
                                        op0=ADD)
                sstd = cp.tile([DIM, 1], f32, name=f"sstd{L}")
                nc.scalar.activation(sstd[:], vare[:], Sqrt, bias=0.0,
                                     scale=1.0)
                rinv = cp.tile([DIM, 1], f32, name=f"rinv{L}")
                nc.vector.reciprocal(rinv[:], sstd[:])
                s_t = cp.tile([DIM, 1], f32, name=f"s{L}")
                nc.vector.tensor_tensor(out=s_t[:], in0=rinv[:],
                                        in1=gamT_t[:, L:L + 1], op=MULT)
                ms = cp.tile([DIM, 1], f32, name=f"ms{L}")
                nc.vector.tensor_tensor(out=ms[:], in0=mean[:], in1=s_t[:],
                                        op=MULT)
                t_t = cp.tile([DIM, 1], f32, name=f"t{L}")
                nc.vector.tensor_tensor(out=t_t[:], in0=betT_t[:, L:L + 1],
                                        in1=ms[:], op=SUB)
                nc.sync.dma_start(st_out.ap()[2 * L, :], s_t[:, 0])
                nc.sync.dma_start(st_out.ap()[2 * L + 1, :], t_t[:, 0])
                # pacc += (s * lcw_L) * pall
                slcw = cp.tile([DIM, 1], f32, name=f"slcw{L}")
                nc.vector.tensor_tensor(out=slcw[:], in0=s_t[:],
                                        in1=lcw_t[:, L:L + 1], op=MULT)
                nc.vector.scalar_tensor_tensor(
                    out=pacc[:], in0=pall[:], scalar=slcw[:, 0:1],
                    in1=pacc[:], op0=MULT, op1=ADD)
                if L < L5 - 1:
                    nc.vector.tensor_scalar(
                        out=w1f[:], in0=w1_t[:, (L + 1) * DIM:(L + 2) * DIM],
                        scalar1=s_t[:], scalar2=None, op0=MULT)
                    vps = psP.tile([DIM, 1], f32, name="vps", tag="pout")
                    nc.tensor.matmul(
                        vps[:], lhsT=w1_t[:, (L + 1) * DIM:(L + 2) * DIM],
                        rhs=t_t[:], start=True, stop=True)
                    vb = cp.tile([DIM, 1], f32r, name=f"vb{L}")
                    nc.vector.tensor_copy(vb[:], vps[:])
                    nc.sync.dma_start(v_dram.ap()[0, :], vb[:, 0])
                    nc.sync.dma_start(vrow[:], v_dram.ap()[:, :])

            nc.sync.dma_start(pacc_out.ap()[:, :], pacc[:])
    nc.compile()
    return nc


def kernel(x, edge_index, batch, percent, ini_w1, ini_b1, ini_w2, ini_b2,
           gin_w1, gin_b1, gin_w2, gin_b2, bn_gamma, bn_beta, lc_w, lc_b):
    x = np.asarray(x, np.float32)
    src = np.asarray(edge_index[0], np.int64)
    dst = np.asarray(edge_index[1], np.int64)
    batch = np.asarray(batch, np.int64)

    percore, T64, PW = _prep(src, dst, batch)

    key = (T64, PW)
    if key not in _CACHE:
        _CACHE[key] = _build(T64, PW)
    nc = _CACHE[key]

    lcw = np.asarray(lc_w, np.float32)
    lcb = np.float32(np.asarray(lc_b, np.float32))
    lcw_pad = np.zeros(8, np.float32)
    lcw_pad[:N_LAYERS] = lcw
    com = dict(
        If=np.eye(128, dtype=np.float32),
        iota=np.tile(np.tile(np.arange(W64, dtype=np.float32), SGRP)
                     [None, :], (128, 1)).astype(ml_dtypes.bfloat16),
        ib2bc=np.tile(np.asarray(ini_b2, np.float32)[None, :], (128, 1)),
        lcwbc=np.tile(lcw_pad[None, :], (128, 1)),
        iw1=np.asarray(ini_w1, np.float32),
        ib1=np.asarray(ini_b1, np.float32).reshape(DIM, 1),
        iw2=np.asarray(ini_w2, np.float32),
        w1=np.concatenate([np.asarray(gin_w1[i], np.float32)
                           for i in range(N_LAYERS)], axis=1),
        b1T=np.asarray(gin_b1, np.float32).T.copy(),
        w2=np.concatenate([np.asarray(gin_w2[i], np.float32)
                           for i in range(N_LAYERS)], axis=1),
        b2r=np.asarray(gin_b2, np.float32).reshape(1, N_LAYERS * DIM),
        gamT=np.asarray(bn_gamma, np.float32).T.copy(),
        betT=np.asarray(bn_beta, np.float32).T.copy(),
    )
    in_maps = []
    for c in range(NC):
        pc = percore[c]
        m = dict(com)
        m["xT"] = x[c * NLOC:(c + 1) * NLOC].T.copy()
        m["idx"] = pc["idx"]
        m["dsl"] = pc["dsl"]
        m["Sp"] = pc["Sp"]
        m["degp"] = pc["degp"]
        in_maps.append(m)

    import os
    trace = os.environ.get("KERNEL_TRACE", "0") == "1"
    res = bass_utils.run_bass_kernel_spmd(
        nc, in_maps, core_ids=list(range(NC)), trace=trace)
    global _LAST_RES
    _LAST_RES = res

    # ---- host unshard/combine
    st = res.results[0]["st"]            # [2L, 128]
    t_all = st[1::2]
    cnt = np.bincount(batch, minlength=N_GRAPHS).astype(np.float32)

    praw = np.zeros((N_GRAPHS, DIM), np.float32)
    for c in range(NC):
        pa = res.results[c]["pacc"]      # [128, NT128*PW]
        glo = percore[c]["glo"]
        for t in range(NT128):
            g0 = int(glo[t])
            w = min(PW - 1, N_GRAPHS - g0)
            praw[g0:g0 + w, :] += pa[:, t * PW:t * PW + w].T

    tvec = (lcw[:, None] * t_all).sum(axis=0)
    out = praw + cnt[:, None] * (tvec[None, :] + lcb)
    return out


# revision 26
# speedup vs baseline: 1.0459x; 1.0459x over previous
"""GIN encoder (5-layer GNN + BN + global pooling) on 8 TRN2 NeuronCores.

kernel(**inputs) takes FULL inputs, returns FULL [8192, 128] output.

Design (SPMD, one bass program, per-core data):
- Nodes in 8 equal contiguous 25000-row shards.
- Per layer: activations all-gathered into a node-major bf16 table
  [200000,128]; per-128-dst-tile aggregation on PE via host-built 0/1
  selection matrices S (gathered src rows via per-chunk indirect DMA with
  OOB-skipped padding), self term from the local prev-z buffer against an
  identity, delayed-BN folded in (scale into W1, shift via rank-1
  (t/s) x deg' matmul).
- MLP1 feature-major, MLP2 node-major (stationary=y1 cols, b2 via K=1 ones
  matmul), relu+mask on ScalarE; BN stats = Gram diagonal + pooled ones
  column, all-reduced; pooling per node-chunk windows, overlap-added on
  host; layer-conv combine on host using device-computed (s_i, t_i).
"""
import numpy as np
import ml_dtypes

from concourse import bass, bacc, tile, mybir
from concourse import bass_utils

N_NODES = 200000
N_EDGES = 400000
N_FEAT = 78
DIM = 128
N_LAYERS = 5
N_GRAPHS = 8192
BN_EPS = 1e-5
NC = 8
NLOC = N_NODES // NC
NT = (NLOC + 127) // 128
OOB = 1 << 30

f32 = mybir.dt.float32
bf16 = mybir.dt.bfloat16
i32 = mybir.dt.int32
Relu = mybir.ActivationFunctionType.Relu
Copy = mybir.ActivationFunctionType.Copy
Sqrt = mybir.ActivationFunctionType.Sqrt
ADD = mybir.AluOpType.add
MULT = mybir.AluOpType.mult
SUB = mybir.AluOpType.subtract

_CACHE = {}
_LAST_RES = None


def _prep(src, dst, batch):
    order = np.argsort(dst, kind="stable")
    src_s = src[order].astype(np.int64)
    dst_s = dst[order].astype(np.int64)

    cores_e = []
    T = 0
    for c in range(NC):
        lo = c * NLOC
        m = (dst_s >= lo) & (dst_s < lo + NLOC)
        es, ed = src_s[m], dst_s[m] - lo
        cnt = np.bincount(ed // 128, minlength=NT)
        T = max(T, int(np.ceil(cnt.max() / 128)))
        cores_e.append((es, ed, cnt))

    PW = 0
    glo_all = []
    for c in range(NC):
        b = batch[c * NLOC:(c + 1) * NLOC]
        glo = np.zeros(NT, np.int64)
        for t in range(NT):
            seg = b[t * 128: min((t + 1) * 128, NLOC)]
            glo[t] = seg[0]
            PW = max(PW, int(seg[-1] - seg[0] + 1))
        glo_all.append(glo)
    PW += 1  # ones column

    percore = []
    for c in range(NC):
        es, ed, cnt = cores_e[c]
        idx = np.full((NT * T, 128), OOB, np.int32)
        S = np.zeros((128, NT * T * 128), np.float32)
        off = np.concatenate([[0], np.cumsum(cnt)])
        for t in range(NT):
            e0, e1 = int(off[t]), int(off[t + 1])
            r = np.arange(e1 - e0)
            idx[t * T + r // 128, r % 128] = es[e0:e1]
            S[r % 128, (t * T + r // 128) * 128 + (ed[e0:e1] - t * 128)] = 1.0

        degp = np.zeros((1, NT * 128), np.float32)
        dcnt = np.bincount(ed, minlength=NLOC).astype(np.float32)
        degp[0, :NLOC] = dcnt + 1.0

        b = batch[c * NLOC:(c + 1) * NLOC]
        Sp = np.zeros((128, NT * PW), np.float32)
        glo = glo_all[c]
        for t in range(NT):
            n0, n1 = t * 128, min(t * 128 + 128, NLOC)
            p = np.arange(n1 - n0)
            Sp[p, t * PW + (b[n0:n1] - glo[t])] = 1.0
            Sp[p, t * PW + PW - 1] = 1.0

        percore.append(dict(
            idx=idx.T.copy(),                      # [128, NCH]
            S=S.astype(ml_dtypes.bfloat16),
            Sp=Sp.astype(ml_dtypes.bfloat16),
            degp=degp.astype(ml_dtypes.bfloat16),
            glo=glo))

    I_full = np.eye(128, dtype=np.float32)
    I_last = np.zeros((128, 128), np.float32)
    base = (NT - 1) * 128 - (NLOC - 128)   # slot offset of first last-tile row
    for j in range(NLOC - (NT - 1) * 128):
        I_last[base + j, j] = 1.0

    mask = np.zeros((128, NT), np.float32)
    for t in range(NT):
        mask[:min(128, NLOC - t * 128), t] = 1.0

    return percore, T, PW, I_full, I_last, mask


def _build(T, PW):
    nc = bacc.Bacc("TRN2", target_bir_lowering=False, debug=False,
                   num_devices=NC)
    NCH = NT * T
    L5 = N_LAYERS

    xT = nc.dram_tensor("xT", [N_FEAT, NLOC], f32, kind="ExternalInput")
    idx_in = nc.dram_tensor("idx", [128, NCH], i32, kind="ExternalInput")
    S_in = nc.dram_tensor("S", [128, NCH * 128], bf16, kind="ExternalInput")
    Sp_in = nc.dram_tensor("Sp", [128, NT * PW], bf16, kind="ExternalInput")
    degp_in = nc.dram_tensor("degp", [1, NT * 128], bf16, kind="ExternalInput")
    If_in = nc.dram_tensor("If", [128, 128], bf16, kind="ExternalInput")
    Il_in = nc.dram_tensor("Il", [128, 128], bf16, kind="ExternalInput")
    mask_in = nc.dram_tensor("mask", [128, NT], f32, kind="ExternalInput")
    iw1_in = nc.dram_tensor("iw1", [N_FEAT, DIM], f32, kind="ExternalInput")
    ib1_in = nc.dram_tensor("ib1", [DIM, 1], f32, kind="ExternalInput")
    iw2_in = nc.dram_tensor("iw2", [DIM, DIM], f32, kind="ExternalInput")
    ib2_in = nc.dram_tensor("ib2", [1, DIM], f32, kind="ExternalInput")
    w1_in = nc.dram_tensor("w1", [DIM, L5 * DIM], f32, kind="ExternalInput")
    w2_in = nc.dram_tensor("w2", [DIM, L5 * DIM], f32, kind="ExternalInput")
    b1T_in = nc.dram_tensor("b1T", [DIM, L5], f32, kind="ExternalInput")
    b2r_in = nc.dram_tensor("b2r", [1, L5 * DIM], f32, kind="ExternalInput")
    gamT_in = nc.dram_tensor("gamT", [DIM, L5], f32, kind="ExternalInput")
    betT_in = nc.dram_tensor("betT", [DIM, L5], f32, kind="ExternalInput")

    pall_out = nc.dram_tensor("pall", [L5, DIM, NT * PW], f32,
                              kind="ExternalOutput")
    st_out = nc.dram_tensor("st", [L5 * 2, DIM], f32, kind="ExternalOutput")

    table = nc.dram_tensor("table", [N_NODES, DIM], bf16, kind="Internal",
                           addr_space="Shared")
    zbuf = [nc.dram_tensor(f"zbuf{i}", [NLOC, DIM], bf16, kind="Internal")
            for i in range(2)]
    ar_in = nc.dram_tensor("ar_in", [DIM, 2], f32, kind="Internal")
    ar_out = nc.dram_tensor("ar_out", [DIM, 2], f32, kind="Internal",
                            addr_space="Shared")
    ts_dram = nc.dram_tensor("ts_dram", [1, DIM], bf16, kind="Internal")
    RG = [list(range(NC))]

    with tile.TileContext(nc) as tc:
        with tc.tile_pool(name="const", bufs=1) as cp, \
             tc.tile_pool(name="gpool", bufs=10) as gpool, \
             tc.tile_pool(name="spool", bufs=5) as spool, \
             tc.tile_pool(name="selfp", bufs=5) as selfp, \
             tc.tile_pool(name="z1p", bufs=2) as z1p, \
             tc.tile_pool(name="y1p", bufs=2) as y1p, \
             tc.tile_pool(name="ztp", bufs=5) as ztp, \
             tc.tile_pool(name="xbp", bufs=2) as xbp, \
             tc.tile_pool(name="pop", bufs=5) as pop, \
             tc.tile_pool(name="psA", bufs=2, space="PSUM") as psA, \
             tc.tile_pool(name="psB", bufs=2, space="PSUM") as psB, \
             tc.tile_pool(name="psC", bufs=1, space="PSUM") as psC, \
             tc.tile_pool(name="psG", bufs=1, space="PSUM") as psG, \
             tc.tile_pool(name="psP", bufs=1, space="PSUM") as psP:

            def ld(shape, dt_, src_ap, name):
                t_ = cp.tile(shape, dt_, name=name)
                nc.sync.dma_start(t_[:], src_ap)
                return t_

            idx_t = ld([128, NCH], i32, idx_in[:], "idx_t")
            Sp_t = ld([128, NT * PW], bf16, Sp_in[:], "Sp_t")
            degp_t = ld([1, NT * 128], bf16, degp_in[:], "degp_t")
            If_t = ld([128, 128], bf16, If_in[:], "If_t")
            Il_t = ld([128, 128], bf16, Il_in[:], "Il_t")
            mask_t = ld([128, NT], f32, mask_in[:], "mask_t")
            iw1_t = ld([N_FEAT, DIM], f32, iw1_in[:], "iw1_t")
            ib1_t = ld([DIM, 1], f32, ib1_in[:], "ib1_t")
            iw2_t = ld([DIM, DIM], f32, iw2_in[:], "iw2_t")
            ib2_t = ld([1, DIM], f32, ib2_in[:], "ib2_t")
            w1_t = ld([DIM, L5 * DIM], f32, w1_in[:], "w1_t")
            w2_t = ld([DIM, L5 * DIM], f32, w2_in[:], "w2_t")
            b1T_t = ld([DIM, L5], f32, b1T_in[:], "b1T_t")
            b2r_t = ld([1, L5 * DIM], f32, b2r_in[:], "b2r_t")
            gamT_t = ld([DIM, L5], f32, gamT_in[:], "gamT_t")
            betT_t = ld([DIM, L5], f32, betT_in[:], "betT_t")

            iw2b = cp.tile([DIM, DIM], bf16, name="iw2b")
            nc.vector.tensor_copy(iw2b[:], iw2_t[:])
            ib2b = cp.tile([1, DIM], bf16, name="ib2b")
            nc.vector.tensor_copy(ib2b[:], ib2_t[:])
            b2rb = cp.tile([1, L5 * DIM], bf16, name="b2rb")
            nc.vector.tensor_copy(b2rb[:], b2r_t[:])
            w2b_all = cp.tile([DIM, L5 * DIM], bf16, name="w2b_all")
            nc.vector.tensor_copy(w2b_all[:], w2_t[:])
            ident_f = cp.tile([128, 128], f32, name="ident_f")
            nc.vector.tensor_copy(ident_f[:], If_t[:])
            ones_r = cp.tile([1, 128], bf16, name="ones_r")
            nc.vector.memset(ones_r[:], 1.0)

            zsum_t = cp.tile([DIM, 1], f32, name="zsum_t")
            stats_sb = cp.tile([DIM, 2], f32, name="stats_sb")
            w1f = cp.tile([DIM, DIM], bf16, name="w1f")
            tsrow = cp.tile([1, DIM], bf16, name="tsrow")
            nc.vector.memset(tsrow[:], 0.0)
            nc.vector.tensor_copy(w1f[:], w1_t[:, 0:DIM])

            for _ in range(10):
                g0 = gpool.tile([128, 128], bf16, name="g", tag="g")
                nc.vector.memset(g0[:], 0.0)

            # ---------------- ini embed -> zbuf[0], table ----------------
            NBL = (NLOC + 511) // 512
            for bblk in range(NBL):
                n0 = bblk * 512
                w = min(512, NLOC - n0)
                xb = xbp.tile([N_FEAT, 512], f32, name="xb", tag="xb")
                nc.sync.dma_start(xb[:, :w], xT[:, n0:n0 + w])
                yp = psB.tile([DIM, 512], f32, name="yp", tag="yp")
                nc.tensor.matmul(yp[:, :w], lhsT=iw1_t[:], rhs=xb[:, :w],
                                 start=True, stop=True)
                y1b = y1p.tile([DIM, 512], bf16, name="y1b", tag="y1")
                nc.scalar.activation(y1b[:, :w], yp[:, :w], Relu,
                                     bias=ib1_t[:], scale=1.0)
                for k in range((w + 127) // 128):
                    cw = min(128, w - k * 128)
                    zp = psC.tile([128, DIM], f32, name="zp", tag="zp")
                    nc.tensor.matmul(zp[:cw, :],
                                     lhsT=y1b[:, k * 128:k * 128 + cw],
                                     rhs=iw2b[:], start=True, stop=False)
                    nc.tensor.matmul(zp[:cw, :], lhsT=ones_r[:, :cw],
                                     rhs=ib2b[:], start=False, stop=True)
                    zt = ztp.tile([128, DIM], bf16, name="zt", tag="zt")
                    nc.scalar.activation(zt[:cw, :], zp[:cw, :], Copy,
                                         scale=1.0)
                    nc.sync.dma_start(
                        zbuf[0].ap()[n0 + k * 128:n0 + k * 128 + cw, :],
                        zt[:cw, :])
            nc.gpsimd.collective_compute(
                "AllGather", mybir.AluOpType.bypass, replica_groups=RG,
                ins=[zbuf[0].ap()], outs=[table.ap()])

            # ---------------- layers ----------------
            for L in range(L5):
                zprev = zbuf[L % 2]
                zcur = zbuf[(L + 1) % 2]
                nc.vector.memset(zsum_t[:], 0.0)
                gram = psG.tile([128, 128], f32, name="gram", tag="gram")
                z1b = None
                for t in range(NT):
                    Sg = spool.tile([128, T * 128], bf16, name="Sg", tag="S")
                    nc.scalar.dma_start(
                        Sg[:], S_in[:, t * T * 128:(t + 1) * T * 128])
                    Gs = selfp.tile([128, 128], bf16, name="Gs", tag="self")
                    r0 = min(t * 128, NLOC - 128)
                    nc.sync.dma_start(Gs[:], zprev.ap()[r0:r0 + 128, :])
                    u = psA.tile([128, 128], f32, name="u", tag="u")
                    for k in range(T):
                        ch = t * T + k
                        g = gpool.tile([128, 128], bf16, name="g", tag="g")
                        nc.gpsimd.indirect_dma_start(
                            out=g[:], out_offset=None, in_=table[:],
                            in_offset=bass.IndirectOffsetOnAxis(
                                ap=idx_t[:, ch:ch + 1], axis=0),
                            bounds_check=N_NODES - 1, oob_is_err=False)
                        nc.tensor.matmul(u[:], lhsT=g[:],
                                         rhs=Sg[:, k * 128:(k + 1) * 128],
                                         start=(k == 0), stop=False)
                    It = Il_t if t == NT - 1 else If_t
                    nc.tensor.matmul(u[:], lhsT=Gs[:], rhs=It[:],
                                     start=False, stop=False)
                    nc.tensor.matmul(u[:], lhsT=tsrow[:],
                                     rhs=degp_t[:, t * 128:(t + 1) * 128],
                                     start=False, stop=True)
                    if t % 4 == 0:
                        z1b = z1p.tile([DIM, 512], bf16, name="z1b", tag="z1")
                    nc.scalar.activation(
                        z1b[:, (t % 4) * 128:(t % 4 + 1) * 128], u[:],
                        Copy, scale=1.0)
                    if t % 4 == 3 or t == NT - 1:
                        bw = (t % 4 + 1) * 128
                        yp = psB.tile([DIM, 512], f32, name="yp", tag="yp")
                        nc.tensor.matmul(yp[:, :bw], lhsT=w1f[:],
                                         rhs=z1b[:, :bw], start=True,
                                         stop=True)
                        y1b = y1p.tile([DIM, 512], bf16, name="y1b", tag="y1")
                        nc.scalar.activation(y1b[:, :bw], yp[:, :bw], Relu,
                                             bias=b1T_t[:, L:L + 1], scale=1.0)
                        for k in range(t % 4 + 1):
                            ck = (t // 4) * 4 + k
                            zp = psC.tile([128, DIM], f32, name="zp", tag="zp")
                            nc.tensor.matmul(
                                zp[:], lhsT=y1b[:, k * 128:(k + 1) * 128],
                                rhs=w2b_all[:, L * DIM:(L + 1) * DIM],
                                start=True, stop=False)
                            nc.tensor.matmul(
                                zp[:], lhsT=ones_r[:],
                                rhs=b2rb[:, L * DIM:(L + 1) * DIM],
                                start=False, stop=True)
                            zt = ztp.tile([128, DIM], bf16, name="zt",
                                          tag="zt")
                            nc.scalar.activation(zt[:], zp[:], Relu,
                                                 scale=mask_t[:, ck:ck + 1])
                            rw = min(128, NLOC - ck * 128)
                            nc.sync.dma_start(
                                zcur.ap()[ck * 128:ck * 128 + rw, :],
                                zt[:rw, :])
                            pout = psP.tile([128, PW], f32, name="pout",
                                            tag="pout")
                            nc.tensor.matmul(
                                pout[:], lhsT=zt[:],
                                rhs=Sp_t[:, ck * PW:(ck + 1) * PW],
                                start=True, stop=True)
                            posb = pop.tile([128, PW], f32, name="posb",
                                            tag="posb")
                            nc.vector.tensor_copy(posb[:], pout[:])
                            nc.vector.tensor_tensor(
                                out=zsum_t[:], in0=zsum_t[:],
                                in1=posb[:, PW - 1:PW], op=ADD)
                            nc.sync.dma_start(
                                pall_out.ap()[L, :, ck * PW:(ck + 1) * PW],
                                posb[:])
                            nc.tensor.matmul(gram[:], lhsT=zt[:], rhs=zt[:],
                                             start=(ck == 0),
                                             stop=(ck == NT - 1))
                # ---- stats + next-layer affine
                gmul = pop.tile([128, 128], f32, name="gmul", tag="gmul")
                nc.vector.tensor_tensor(out=gmul[:], in0=gram[:],
                                        in1=ident_f[:], op=MULT)
                zsq = cp.tile([DIM, 1], f32, name=f"zsq{L}")
                nc.vector.tensor_reduce(out=zsq[:], in_=gmul[:], op=ADD,
                                        axis=mybir.AxisListType.X)
                nc.vector.tensor_copy(stats_sb[:, 0:1], zsum_t[:])
                nc.vector.tensor_copy(stats_sb[:, 1:2], zsq[:])
                nc.sync.dma_start(ar_in.ap()[:, :], stats_sb[:])
                nc.gpsimd.collective_compute(
                    "AllReduce", ADD, replica_groups=RG,
                    ins=[ar_in.ap()], outs=[ar_out.ap()])
                if L < L5 - 1:
                    nc.gpsimd.collective_compute(
                        "AllGather", mybir.AluOpType.bypass, replica_groups=RG,
                        ins=[zcur.ap()], outs=[table.ap()])
                arst = cp.tile([DIM, 2], f32, name=f"arst{L}")
                nc.sync.dma_start(arst[:], ar_out.ap()[:, :])
                mean = cp.tile([DIM, 1], f32, name=f"mean{L}")
                nc.vector.tensor_scalar(out=mean[:], in0=arst[:, 0:1],
                                        scalar1=1.0 / N_NODES, scalar2=None,
                                        op0=MULT)
                ex2 = cp.tile([DIM, 1], f32, name=f"ex2{L}")
                nc.vector.tensor_scalar(out=ex2[:], in0=arst[:, 1:2],
                                        scalar1=1.0 / N_NODES, scalar2=None,
                                        op0=MULT)
                m2 = cp.tile([DIM, 1], f32, name=f"m2{L}")
                nc.vector.tensor_tensor(out=m2[:], in0=mean[:], in1=mean[:],
                                        op=MULT)
                var = cp.tile([DIM, 1], f32, name=f"var{L}")
                nc.vector.tensor_tensor(out=var[:], in0=ex2[:], in1=m2[:],
                                        op=SUB)
                vare = cp.tile([DIM, 1], f32, name=f"vare{L}")
                nc.vector.tensor_scalar(out=vare[:], in0=var[:],
                                        scalar1=BN_EPS, scalar2=None,
                                        op0=ADD)
                sstd = cp.tile([DIM, 1], f32, name=f"sstd{L}")
                nc.scalar.activation(sstd[:], vare[:], Sqrt, bias=0.0,
                                     scale=1.0)
                rinv = cp.tile([DIM, 1], f32, name=f"rinv{L}")
                nc.vector.reciprocal(rinv[:], sstd[:])
                s_t = cp.tile([DIM, 1], f32, name=f"s{L}")
                nc.vector.tensor_tensor(out=s_t[:], in0=rinv[:],
                                        in1=gamT_t[:, L:L + 1], op=MULT)
                ms = cp.tile([DIM, 1], f32, name=f"ms{L}")
                nc.vector.tensor_tensor(out=ms[:], in0=mean[:], in1=s_t[:],
                                        op=MULT)
                t_t = cp.tile([DIM, 1], f32, name=f"t{L}")
                nc.vector.tensor_tensor(out=t_t[:], in0=betT_t[:, L:L + 1],
                                        in1=ms[:], op=SUB)
                nc.sync.dma_start(st_out.ap()[2 * L, :], s_t[:, 0])
                nc.sync.dma_start(st_out.ap()[2 * L + 1, :], t_t[:, 0])
                if L < L5 - 1:
                    rs = cp.tile([DIM, 1], f32, name=f"rs{L}")
                    nc.vector.reciprocal(rs[:], s_t[:])
                    tsf = cp.tile([DIM, 1], f32, name=f"tsf{L}")
                    nc.vector.tensor_tensor(out=tsf[:], in0=t_t[:],
                                            in1=rs[:], op=MULT)
                    tsb = cp.tile([DIM, 1], bf16, name=f"tsb{L}")
                    nc.vector.tensor_copy(tsb[:], tsf[:])
                    nc.sync.dma_start(ts_dram.ap()[0, :], tsb[:, 0])
                    nc.sync.dma_start(tsrow[:], ts_dram.ap()[:, :])
                    nc.vector.tensor_scalar(
                        out=w1f[:], in0=w1_t[:, (L + 1) * DIM:(L + 2) * DIM],
                        scalar1=s_t[:], scalar2=None, op0=MULT)
    nc.compile()
    return nc


def kernel(x, edge_index, batch, percent, ini_w1, ini_b1, ini_w2, ini_b2,
           gin_w1, gin_b1, gin_w2, gin_b2, bn_gamma, bn_beta, lc_w, lc_b):
    x = np.asarray(x, np.float32)
    src = np.asarray(edge_index[0], np.int64)
    dst = np.asarray(edge_index[1], np.int64)
    batch = np.asarray(batch, np.int64)

    percore, T, PW, I_full, I_last, mask = _prep(src, dst, batch)

    key = (T, PW)
    if key not in _CACHE:
        _CACHE[key] = _build(T, PW)
    nc = _CACHE[key]

    bf = ml_dtypes.bfloat16
    com = dict(
        If=I_full.astype(bf), Il=I_last.astype(bf), mask=mask,
        iw1=np.asarray(ini_w1, np.float32),
        ib1=np.asarray(ini_b1, np.float32).reshape(DIM, 1),
        iw2=np.asarray(ini_w2, np.float32),
        ib2=np.asarray(ini_b2, np.float32).reshape(1, DIM),
        w1=np.concatenate([np.asarray(gin_w1[i], np.float32)
                           for i in range(N_LAYERS)], axis=1),
        w2=np.concatenate([np.asarray(gin_w2[i], np.float32)
                           for i in range(N_LAYERS)], axis=1),
        b1T=np.asarray(gin_b1, np.float32).T.copy(),
        b2r=np.asarray(gin_b2, np.float32).reshape(1, N_LAYERS * DIM),
        gamT=np.asarray(bn_gamma, np.float32).T.copy(),
        betT=np.asarray(bn_beta, np.float32).T.copy(),
    )
    in_maps = []
    for c in range(NC):
        pc = percore[c]
        m = dict(com)
        m["xT"] = x[c * NLOC:(c + 1) * NLOC].T.copy()
        m["idx"] = pc["idx"]
        m["S"] = pc["S"]
        m["Sp"] = pc["Sp"]
        m["degp"] = pc["degp"]
        in_maps.append(m)

    import os
    trace = os.environ.get("KERNEL_TRACE", "0") == "1"
    res = bass_utils.run_bass_kernel_spmd(
        nc, in_maps, core_ids=list(range(NC)), trace=trace)
    global _LAST_RES
    _LAST_RES = res

    # ---- host unshard/combine
    lcw = np.asarray(lc_w, np.float32)
    lcb = np.float32(np.asarray(lc_b, np.float32))
    st = res.results[0]["st"]            # [2L, 128]
    s_all = st[0::2]                     # [L, 128]
    t_all = st[1::2]
    cnt = np.bincount(batch, minlength=N_GRAPHS).astype(np.float32)

    praw = np.zeros((N_LAYERS, N_GRAPHS, DIM), np.float32)
    for c in range(NC):
        pall = res.results[c]["pall"]    # [L, 128, NT*PW]
        glo = percore[c]["glo"]
        for t in range(NT):
            w = PW - 1
            g0 = int(glo[t])
            w = min(w, N_GRAPHS - g0)
            praw[:, g0:g0 + w, :] += pall[:, :, t * PW:t * PW + w].transpose(
                0, 2, 1)

    out = np.zeros((N_GRAPHS, DIM), np.float32)
    for i in range(N_LAYERS):
        out += lcw[i] * (praw[i] * s_all[i][None, :]
                         + cnt[:, None] * t_all[i][None, :])
    out += lcb * cnt[:, None]
    return out

